# revision 33
# baseline (speedup 1.0000x reference)
"""ASENet_V2 forward pass on 8 Trainium2 NeuronCores, data-parallel over batch.

Per core (64 samples):
  - conv1x1+BN folded on host -> img = tanh(Wc.T @ x); half the K dim runs
    as fp8e4 DoubleRow matmuls (2 k-chunks per instruction, quad-pumped),
    the other half fp16.  All conv operands share one power-of-2 scale so
    the mixed PSUM accumulation is descaled exactly inside the tanh.
  - attention logits via M=1 matmuls into a [2,196] PSUM tile (one
    partition per sample of the group); batched max/exp per group.
  - attended feature via one pass-wide (4 samples) DVE multiply + reduce.
  - gated-fusion MLP split in two 32-sample halves; the first half
    overlaps the conv of the second half of the batch.
  - l2-norm via ones-matmul partition reduction.
Returns full [512, 1024] output.
"""
import sys

sys.path.insert(0, "/opt/trn_rl_repo")

import ml_dtypes
import numpy as np

import concourse.bass as bass
import concourse.tile as tile
from concourse import bacc, mybir
from concourse.bass_utils import run_bass_kernel_spmd

F32 = mybir.dt.float32
F16 = mybir.dt.float16
F8 = mybir.dt.float8e4
AF = mybir.ActivationFunctionType
ALU = mybir.AluOpType
AX = mybir.AxisListType
PM = mybir.MatmulPerfMode

B, C_IN, C_MID, HW2, EMB = 512, 1024, 512, 196, 1024
N_CORES = 8
NS = B // N_CORES          # samples per core = 64
SP = 4                     # samples per DMA pass
NG = 2                     # samples per matmul group (N = 392)
N_PASS = NS // SP          # 16
NGR = NS // NG             # 32 groups
BN_EPS = 1e-5

FP8_PAIRS = 2              # k-chunk pairs (256 ch each) of conv K in fp8
N8 = FP8_PAIRS * 2         # fp8 k-chunks (128 ch each)
N16 = 8 - N8               # fp16 k-chunks
SX = 32.0                  # fp8 x scale (power of 2)
SW = 1024.0                # fp8 W scale (power of 2)
S16 = SX * SW if N8 else 1.0
DESCALE = 1.0 / S16

_NC_CACHE = {}


def build_nc():
    nc = bacc.Bacc("TRN2", target_bir_lowering=False, debug=False)

    # ---- DRAM I/O (per-core shapes); all weights packed [p, (chunk, col)]
    x16_d = nc.dram_tensor("x16", [N_PASS, 128, 8 * SP * HW2], F16,
                           kind="ExternalInput").ap()
    if N8:
        x8_d = nc.dram_tensor("x8", [N_PASS, 128, N8 * SP * HW2], F8,
                              kind="ExternalInput").ap()
        wc8_d = nc.dram_tensor("wc8", [128, N8 * C_MID], F8,
                               kind="ExternalInput").ap()
    wct16_d = nc.dram_tensor("wct16", [128, N16 * C_MID], F16,
                             kind="ExternalInput").ap()
    bc_d = nc.dram_tensor("bc", [128, 4], F32, kind="ExternalInput").ap()
    a1_d = nc.dram_tensor("a1", [128, 4 * NS], F16, kind="ExternalInput").ap()
    a2_d = nc.dram_tensor("a2", [128, 4 * NS], F16, kind="ExternalInput").ap()
    w1_d = nc.dram_tensor("w1", [128, 12 * 512], F16, kind="ExternalInput").ap()
    b1_d = nc.dram_tensor("b1", [128, 4], F32, kind="ExternalInput").ap()
    w2_d = nc.dram_tensor("w2", [128, 4 * 1024], F16, kind="ExternalInput").ap()
    b2_d = nc.dram_tensor("b2", [128, 8], F32, kind="ExternalInput").ap()
    wf_d = nc.dram_tensor("wf", [128, 8 * 1024], F16, kind="ExternalInput").ap()
    bf_d = nc.dram_tensor("bf", [128, 8], F32, kind="ExternalInput").ap()
    outT_d = nc.dram_tensor("outT", [8, 128, NS], F32, kind="ExternalOutput").ap()

    with tile.TileContext(nc) as tc:
        with tc.tile_pool(name="persist", bufs=1) as pp:
            wct16 = pp.tile([128, N16 * C_MID], F16)
            wc8 = None
            if N8:
                wc8 = pp.tile([128, N8 * C_MID], F8)
            bc_t = pp.tile([128, 4], F32)
            a1t = pp.tile([128, 4 * NS], F16)
            w1t = pp.tile([128, 12 * 512], F16)
            w2t = pp.tile([128, 4 * 1024], F16)
            wft = pp.tile([128, 8 * 1024], F16)
            b1t = pp.tile([128, 4], F32)
            b2t = pp.tile([128, 8], F32)
            bft = pp.tile([128, 8], F32)
            ones = pp.tile([128, 1], F32)
            nbias = pp.tile([1, 1], F32)          # fixed exp bias
            nc.vector.memset(nbias[:], -4.0)
            Fu = pp.tile([128, 8 * NS], F32)      # feat unnormalized [p,(k,s)]
            Fu16 = pp.tile([128, 8 * NS], F16)    # normalized feat [p,(k,s)]
            a2t = pp.tile([128, 4 * NS], F16)     # relu(Wt2@emb) table [p,(k,s)]
            ssum = pp.tile([1, NS], F32)          # per-sample sum(exp)

            # critical weights first (scalar queue): conv needs these
            if N8:
                nc.scalar.dma_start(wc8[:], wc8_d)
            nc.scalar.dma_start(wct16[:], wct16_d)
            nc.scalar.dma_start(bc_t[:], bc_d)
            nc.scalar.dma_start(a1t[:], a1_d)
            nc.vector.memset(ones[:], 1.0)

            with tc.tile_pool(name="xt16", bufs=4) as xp16, \
                 tc.tile_pool(name="xt8", bufs=3) as xp8, \
                 tc.tile_pool(name="img", bufs=16) as ip, \
                 tc.tile_pool(name="bt", bufs=3) as btp, \
                 tc.tile_pool(name="ex", bufs=4) as exp_, \
                 tc.tile_pool(name="scr", bufs=3) as scrp, \
                 tc.tile_pool(name="prod", bufs=4) as prodp, \
                 tc.tile_pool(name="mlp", bufs=2) as mp, \
                 tc.tile_pool(name="convps", bufs=5, space="PSUM") as cps, \
                 tc.tile_pool(name="attps", bufs=2, space="PSUM") as aps, \
                 tc.tile_pool(name="mlpps", bufs=1, space="PSUM") as mps:

                def issue_x(p, split=False):
                    xt = xp16.tile([128, 8 * SP * HW2], F16, tag="xt16")
                    x8t = None
                    if N8:
                        x8t = xp8.tile([128, N8 * SP * HW2], F8, tag="xt8")
                        nc.sync.dma_start(x8t[:], x8_d[p])
                    if split:
                        v = xt[:].rearrange("p (k s h) -> p k s h", k=8, s=SP)
                        d = x16_d[p].rearrange("p (k s h) -> p k s h", k=8, s=SP)
                        for g2 in range(SP // NG):
                            nc.sync.dma_start(v[:, :, g2 * NG:(g2 + 1) * NG, :],
                                              d[:, :, g2 * NG:(g2 + 1) * NG, :])
                    else:
                        nc.sync.dma_start(xt[:], x16_d[p])
                    return xt, x8t

                def emit_attention(gg, imgs, bt, xt):
                    g2 = gg % (SP // NG)
                    lp = aps.tile([1, NG * HW2], F32, tag="attps")
                    for r in range(NG):
                        s = gg * NG + r
                        for kt in range(4):
                            nc.tensor.matmul(
                                lp[0:1, r * HW2:(r + 1) * HW2],
                                a1t[:, kt * NS + s:kt * NS + s + 1],
                                imgs[kt][:, r * HW2:(r + 1) * HW2],
                                start=(kt == 0), stop=(kt == 3))
                    # logits are tanh(.)-bounded and near N(0,~0.3); a fixed
                    # bias keeps exp in fp16 range without a max pass
                    ex = exp_.tile([1, NG * HW2], F16, tag="ex")
                    fu_v = Fu[:].rearrange("p (k s) -> p k s", k=8)
                    x4 = xt[:].rearrange("p (k s h) -> p k s h", k=8, s=SP)
                    for r in range(NG):
                        s = gg * NG + r
                        sl = g2 * NG + r
                        nc.scalar.activation(
                            ex[0:1, r * HW2:(r + 1) * HW2],
                            lp[0:1, r * HW2:(r + 1) * HW2],
                            AF.Exp, bias=nbias[:],
                            accum_out=ssum[0:1, s:s + 1])
                        bs = bt[:, sl * HW2:(sl + 1) * HW2]
                        nc.gpsimd.partition_broadcast(
                            bs, ex[0:1, r * HW2:(r + 1) * HW2])
                        # attended feature: per-sample multiply into a
                        # contiguous tile + fp32 segmented reduce (the two
                        # DVE patterns measured fastest on HW)
                        prod = prodp.tile([128, 8 * HW2], F16, tag="prod")
                        nc.vector.tensor_tensor(
                            prod[:].rearrange("p (k h) -> p k h", k=8),
                            x4[:, :, sl, :],
                            bs.rearrange("p h -> p () h")
                              .broadcast_to([128, 8, HW2]),
                            op=ALU.mult)
                        nc.vector.tensor_reduce(
                            fu_v[:, 0:8, s:s + 1],
                            prod[:].rearrange("p (k h) -> p k h", k=8),
                            axis=AX.X, op=ALU.add)

                def emit_mlp_half(h):
                    sc = slice(h * 32, (h + 1) * 32)
                    rec = mp.tile([1, 32], F32, tag="rec")
                    nc.vector.reciprocal(rec[:], ssum[0:1, sc])
                    recb = mp.tile([128, 32], F32, tag="recb")
                    nc.gpsimd.partition_broadcast(recb[:], rec[0:1, :])
                    fz = Fu[:].rearrange("p (k s) -> p k s", k=8)
                    f16v = Fu16[:].rearrange("p (k s) -> p k s", k=8)
                    nc.vector.tensor_tensor(
                        f16v[:, :, sc], fz[:, :, sc],
                        recb[:].rearrange("p s -> p () s")
                               .broadcast_to([128, 8, 32]),
                        op=ALU.mult)
                    # h1 = relu(W1 @ [feat; a2] + b1): [512, 32]
                    h1p = mps.tile([128, 8 * 32], F32, tag="mlpps")
                    h1 = mp.tile([128, 4 * 32], F16, tag="h1")
                    for mt in range(4):
                        for kt in range(12):
                            rhs = (Fu16[:, kt * NS + h * 32:kt * NS + h * 32 + 32]
                                   if kt < 8 else
                                   a2t[:, (kt - 8) * NS + h * 32:
                                       (kt - 8) * NS + h * 32 + 32])
                            nc.tensor.matmul(
                                h1p[:, mt * 32:(mt + 1) * 32],
                                w1t[:, kt * 512 + mt * 128:kt * 512 + (mt + 1) * 128],
                                rhs,
                                start=(kt == 0), stop=(kt == 11))
                        nc.scalar.activation(h1[:, mt * 32:(mt + 1) * 32],
                                             h1p[:, mt * 32:(mt + 1) * 32],
                                             AF.Relu, bias=b1t[:, mt:mt + 1])
                    # mask = sigmoid(W2 @ h1 + b2); g = feat * mask
                    mkp = mps.tile([128, 8 * 32], F32, tag="mlpps")
                    msk = mp.tile([128, 8 * 32], F16, tag="msk")
                    for mt in range(8):
                        for kt in range(4):
                            nc.tensor.matmul(
                                mkp[:, mt * 32:(mt + 1) * 32],
                                w2t[:, kt * 1024 + mt * 128:kt * 1024 + (mt + 1) * 128],
                                h1[:, kt * 32:(kt + 1) * 32],
                                start=(kt == 0), stop=(kt == 3))
                        nc.scalar.activation(msk[:, mt * 32:(mt + 1) * 32],
                                             mkp[:, mt * 32:(mt + 1) * 32],
                                             AF.Sigmoid, bias=b2t[:, mt:mt + 1])
                    gg16 = mp.tile([128, 8 * 32], F16, tag="gg")
                    g3 = gg16[:].rearrange("p (k s) -> p k s", k=8)
                    m3 = msk[:].rearrange("p (k s) -> p k s", k=8)
                    nc.vector.tensor_tensor(g3, f16v[:, 0:8, sc], m3, op=ALU.mult)
                    # out = Wf @ g + bf; sq = out^2
                    oop = mps.tile([128, 8 * 32], F32, tag="mlpps")
                    oo = mp.tile([128, 8 * 32], F32, tag="oo")
                    sq = mp.tile([128, 8 * 32], F32, tag="sq")
                    for mt in range(8):
                        for kt in range(8):
                            nc.tensor.matmul(
                                oop[:, mt * 32:(mt + 1) * 32],
                                wft[:, kt * 1024 + mt * 128:kt * 1024 + (mt + 1) * 128],
                                gg16[:, kt * 32:(kt + 1) * 32],
                                start=(kt == 0), stop=(kt == 7))
                        nc.scalar.activation(oo[:, mt * 32:(mt + 1) * 32],
                                             oop[:, mt * 32:(mt + 1) * 32],
                                             AF.Identity, bias=bft[:, mt:mt + 1])
                    nc.scalar.activation(sq[:], oo[:], AF.Square)
                    # l2 norm over channels (partitions x 8 chunks)
                    npt_t = mps.tile([128, 8 * 32], F32, tag="mlpps")
                    npt = npt_t[0:1, 0:32]
                    for kt in range(8):
                        nc.tensor.matmul(npt, ones[:],
                                         sq[:, kt * 32:(kt + 1) * 32],
                                         start=(kt == 0), stop=(kt == 7))
                    nrm = mp.tile([1, 32], F32, tag="nrm")
                    nc.scalar.sqrt(nrm[:], npt)
                    inv = mp.tile([1, 32], F32, tag="inv")
                    nc.vector.reciprocal(inv[:], nrm[:])
                    invb = mp.tile([128, 32], F32, tag="invb")
                    nc.gpsimd.partition_broadcast(invb[:], inv[:])
                    res = mp.tile([128, 8 * 32], F32, tag="res")
                    nc.vector.tensor_tensor(
                        res[:].rearrange("p (k s) -> p k s", k=8),
                        oo[:].rearrange("p (k s) -> p k s", k=8),
                        invb[:].rearrange("p s -> p () s")
                               .broadcast_to([128, 8, 32]),
                        op=ALU.mult)
                    nc.scalar.dma_start(
                        outT_d.rearrange("m p s -> p m s")[:, :, sc],
                        res[:].rearrange("p (m s) -> p m s", m=8))

                xts = {}
                xts[0] = issue_x(0, split=True)
                xts[1] = issue_x(1)
                pend_att = []

                for p in range(N_PASS):
                    if p + 2 < N_PASS:
                        xts[p + 2] = issue_x(p + 2)
                    # non-critical weights spread over early passes (scalar q)
                    if p == 1:
                        nc.gpsimd.dma_start(a2t[:], a2_d)
                        nc.gpsimd.dma_start(b1t[:], b1_d)
                        nc.gpsimd.dma_start(b2t[:], b2_d)
                        nc.gpsimd.dma_start(bft[:], bf_d)
                    elif p == 2:
                        nc.gpsimd.dma_start(w1t[:], w1_d)
                    elif p == 3:
                        nc.gpsimd.dma_start(w2t[:], w2_d)
                    elif p == 4:
                        nc.gpsimd.dma_start(wft[:], wf_d)

                    xt, x8t = xts[p]
                    bt = btp.tile([128, SP * HW2], F16, tag="bt")
                    if N8:
                        x8v = x8t[:].rearrange("p (pr two s h) -> p pr two (s h)",
                                               pr=FP8_PAIRS, two=2, s=SP)
                        w8v = wc8[:].rearrange("p (pr two m) -> p pr two m",
                                               pr=FP8_PAIRS, two=2)
                    for g2 in range(SP // NG):
                        gg = p * (SP // NG) + g2
                        imgs = []
                        for mt in range(4):
                            cpt = cps.tile([128, NG * HW2], F32, tag="convps")
                            first = True
                            if N8:
                                for pr in range(FP8_PAIRS):
                                    nc.tensor.matmul(
                                        cpt[:],
                                        w8v[:, pr, :, mt * 128:(mt + 1) * 128],
                                        x8v[:, pr, :,
                                            g2 * NG * HW2:(g2 + 1) * NG * HW2],
                                        start=first, stop=False,
                                        perf_mode=PM.DoubleRow)
                                    first = False
                            for k in range(N16):
                                rhs = xt[:, ((N8 + k) * SP + g2 * NG) * HW2:
                                         ((N8 + k) * SP + (g2 + 1) * NG) * HW2]
                                nc.tensor.matmul(
                                    cpt[:],
                                    wct16[:, k * C_MID + mt * 128:
                                          k * C_MID + (mt + 1) * 128],
                                    rhs, start=first, stop=(k == N16 - 1))
                                first = False
                            im = ip.tile([128, NG * HW2], F16, tag="img")
                            nc.scalar.activation(im[:], cpt[:], AF.Tanh,
                                                 bias=bc_t[:, mt:mt + 1],
                                                 scale=DESCALE)
                            imgs.append(im)
                        pend_att.append((gg, imgs, bt, xt))
                        if len(pend_att) > 2:
                            emit_attention(*pend_att.pop(0))
                    if p == 8:
                        emit_mlp_half(0)

                while pend_att:
                    emit_attention(*pend_att.pop(0))
                emit_mlp_half(1)

    nc.compile()
    return nc


def prep_inputs(x, c, attr_emb, Wt1, bt1, Wc, bc, bn_gamma, bn_beta, bn_mean,
                bn_var, Wt2, bt2, W1, b1, W2, b2, Wf, bf):
    """Host-side prep: fold BN, build attr tables, quantize, per-core shard."""
    x = np.asarray(x, dtype=np.float32).reshape(B, C_IN, HW2)
    c = np.asarray(c).astype(np.int64)

    scale = np.asarray(bn_gamma) / np.sqrt(np.asarray(bn_var) + BN_EPS)
    Wc_f = (np.asarray(Wc) * scale[:, None]).astype(np.float32)      # [512,1024]
    bc_f = ((np.asarray(bc) - np.asarray(bn_mean)) * scale
            + np.asarray(bn_beta)).astype(np.float32)

    # x16: [cores, pass, 128, (k8, s4, hw)] fp16 (unscaled; shared w/ feat)
    xp = x.reshape(N_CORES, N_PASS, SP, 8, 128, HW2).transpose(0, 1, 4, 3, 2, 5)
    x16 = np.ascontiguousarray(xp, dtype=np.float16).reshape(
        N_CORES, N_PASS, 128, 8 * SP * HW2)

    if N8:
        x8c = np.clip(x[:, :N8 * 128] * SX, -240, 240).astype(
            ml_dtypes.float8_e4m3)
        x8p = x8c.reshape(N_CORES, N_PASS, SP, FP8_PAIRS, 2, 128, HW2) \
                 .transpose(0, 1, 5, 3, 4, 2, 6)
        x8 = np.ascontiguousarray(x8p).reshape(
            N_CORES, N_PASS, 128, N8 * SP * HW2)
        w8 = np.clip(Wc_f[:, :N8 * 128].T * SW, -240, 240).astype(
            ml_dtypes.float8_e4m3)                                   # [k, m]
        wc8 = np.ascontiguousarray(
            w8.reshape(FP8_PAIRS, 2, 128, C_MID).transpose(2, 0, 1, 3)
        ).reshape(128, N8 * C_MID)

    w16 = (Wc_f[:, N8 * 128:].T * S16).astype(np.float16)            # [k, m]
    wct16 = np.ascontiguousarray(
        w16.reshape(N16, 128, C_MID).transpose(1, 0, 2)).reshape(128, N16 * C_MID)
    bc_t = np.ascontiguousarray(bc_f.reshape(4, 128).T)              # [128, 4]

    emb_tab = np.asarray(attr_emb, dtype=np.float32)                 # [8, 512]
    a1_tab = np.tanh(emb_tab @ np.asarray(Wt1).T + np.asarray(bt1))
    a1_tab = (a1_tab / np.sqrt(512.0)).astype(np.float32)
    a2_tab = np.maximum(emb_tab @ np.asarray(Wt2).T + np.asarray(bt2), 0.0)
    a2_tab = a2_tab.astype(np.float32)

    def pack_w(W, nk):                                               # [out, in]
        wT = np.asarray(W, dtype=np.float16).T                       # [in, out]
        m = wT.shape[1]
        return np.ascontiguousarray(
            wT.reshape(nk, 128, m).transpose(1, 0, 2)).reshape(128, nk * m)

    w1p = pack_w(W1, 12)
    w2p = pack_w(W2, 4)
    wfp = pack_w(Wf, 8)
    b1_t = np.ascontiguousarray(np.asarray(b1, np.float32).reshape(4, 128).T)
    b2_t = np.ascontiguousarray(np.asarray(b2, np.float32).reshape(8, 128).T)
    bf_t = np.ascontiguousarray(np.asarray(bf, np.float32).reshape(8, 128).T)

    in_maps = []
    for core in range(N_CORES):
        sl = slice(core * NS, (core + 1) * NS)
        a1 = a1_tab[c[sl]]                                           # [64, 512]
        a2 = a2_tab[c[sl]]
        im = {
            "x16": x16[core],
            "wct16": wct16, "bc": bc_t,
            "a1": np.ascontiguousarray(
                a1.T.reshape(4, 128, NS).transpose(1, 0, 2)
            ).reshape(128, 4 * NS).astype(np.float16),
            "a2": np.ascontiguousarray(
                a2.T.reshape(4, 128, NS).transpose(1, 0, 2)
            ).reshape(128, 4 * NS).astype(np.float16),
            "w1": w1p, "b1": b1_t, "w2": w2p, "b2": b2_t,
            "wf": wfp, "bf": bf_t,
        }
        if N8:
            im["x8"] = x8[core]
            im["wc8"] = wc8
        in_maps.append(im)
    return in_maps


def kernel(**inputs):
    if "nc" not in _NC_CACHE:
        _NC_CACHE["nc"] = build_nc()
    nc = _NC_CACHE["nc"]
    in_maps = prep_inputs(**inputs)
    res = run_bass_kernel_spmd(nc, in_maps, core_ids=list(range(N_CORES)))
    outs = []
    for core in range(N_CORES):
        o = res.results[core]["outT"].reshape(EMB, NS)               # [1024, 64]
        outs.append(np.ascontiguousarray(o.T))                       # [64, 1024]
    return np.concatenate(outs, axis=0).astype(np.float32)           # [512, 1024]
